# revision 18
# baseline (speedup 1.0000x reference)
"""Trainium2 Bass kernel for nn_AttnMLP: 4x (LayerNorm -> Linear(2048,2048) -> tanh-GELU).

Sharding: data-parallel, batch dim (8 batch elements) across 8 NeuronCores.

v2 design (vs the fp32r baseline):
  - bf16 activations+weights (matmul rate is 1 cyc/row for both fp32r and
    bf16 on TRN2, but bf16 halves DMA traffic and SBUF footprint).
  - Activations SBUF-resident across all 4 layers: 16 slots of [128, 2048]
    bf16, written in place by the epilogue (no DRAM round-trips).
  - x-transposes moved off the PE onto the DMA XBAR: one dma_start_transpose
    per token tile (SBUF->SBUF, bf16) produces xT [128, 16, 128] directly.
    PE stream is pure matmul (~874us/pass floor at 2.4GHz).
  - Weights streamed per layer in bf16, double-buffered (prefetched one
    layer ahead, one chunk per token tile, on the Act HWDGE queue; the
    latency-critical transposes have the SP HWDGE queue to themselves).
  - Bias broadcast [128, 2048] precomputed on host (brep input).
  - LN affine (ln_w, ln_b) folded into W and b on the host:
      W' = W * ln_w[None, :],  b' = b + W @ ln_b
  - Output written bf16, upcast to fp32 on host.
  - The tile finalizer splits bf16 matmuls into Ldweights+Matmult pairs;
    _elide_ldweights drops the 3/4 redundant loads (lhsT shared across the
    4 e-chunk matmuls), leaving the structural 1-per-(tile,k) load.
  - Final tile runs e-outer (each PSUM group completes early; its
    add+gelu+store overlaps the remaining matmuls) to shorten the drain
    tail; costs the LDW elision on that one tile only.
  - lookahead=2 (clamped to NT-1): prep runs two tiles ahead so real-HW
    jitter in the DVE/XBAR prep chain cannot stall the PE.

Measured (8-core, axon, min-statistic repeat-slope): ~880-930 us
steady-state per 4-layer pass (vs ~1170-1240 us for the fp32r
DRAM-round-trip baseline, same estimator) — at the structural floor:
872 us matmul rows (1 cyc/row @ 2.4 GHz) + ~56 us LDWEIGHTS. Max rel err
1.06e-2 vs the fp32 reference (bf16 quantization, ~0.26%/layer).

Per-core per-tile pipeline (software-pipelined `lookahead` tiles ahead):
  prep(i):  DVE bn_stats/bn_aggr -> rstd (ScalarE sqrt, DVE recip);
            DVE in-place normalize of resident slot; SP dma_start_transpose
            slot -> xT[128, 16, 128]
  matmul(i): for k in 16: lhsT=xT[:,k,:] (stationary, LDW elided over e);
             for e in 4: PSUM[e] += xT_k.T @ W_k[:, 512e] (bf16, 512 rows)
  epilogue(i): DVE tensor_add (PSUM + brep -> slot, bf16), ScalarE
             Gelu_apprx_tanh in place; last layer: DMA slot -> y.
"""

import sys

sys.path.insert(0, "/opt/trn_rl_repo")

import numpy as np

N_LAYERS = 4
D = 2048  # embedding dim
B = 8  # batch (one element per core)
S = 2048  # sequence length
T = S  # tokens per core
P = 128  # partitions
KC = D // P  # 16 contraction chunks
EC = 4  # output-feature chunks (PSUM banks per tile)
EW = D // EC  # 512 output features per chunk
LN_EPS = 1e-5


def build(nc, T_tokens=T, n_layers=N_LAYERS, repeat=1, lookahead=2,
          transpose_mode="dma"):
    """Emit the kernel IR into `nc`.

    repeat > 1 re-runs the whole n_layers stack that many times (reusing the
    same weights) — a timing-only amplifier for slope measurements."""
    import concourse.bass as bass
    import concourse.mybir as mybir
    import concourse.tile as tile
    from contextlib import ExitStack

    f32 = mybir.dt.float32
    bf16 = mybir.dt.bfloat16
    NT = T_tokens // P  # token tiles
    # prep(j+lookahead) reads slot (j+lookahead) % NT, which epilogue(j)
    # has not yet written when the slot index wraps -> lookahead < NT.
    lookahead = min(lookahead, NT - 1) if NT > 1 else 0
    n_steps = n_layers * repeat

    x_d = nc.dram_tensor("x", [T_tokens, D], bf16, kind="ExternalInput")
    wt_d = nc.dram_tensor("wt", [n_layers, D, D], bf16, kind="ExternalInput")
    br_d = nc.dram_tensor("brep", [n_layers, P, D], bf16, kind="ExternalInput")
    y_d = nc.dram_tensor("y", [T_tokens, D], bf16, kind="ExternalOutput")

    x_v = x_d.rearrange("(n p) d -> n p d", p=P)
    y_v = y_d.rearrange("(n p) d -> n p d", p=P)
    wt_v = wt_d.rearrange("l (kc p) e -> l kc p e", p=P)  # [L, 16, 128, 2048]

    with tile.TileContext(nc) as tc, ExitStack() as ctx:
        singles = ctx.enter_context(tc.tile_pool(name="singles", bufs=1))
        xres_pool = ctx.enter_context(tc.tile_pool(name="xres", bufs=1))
        # chunks 0-1 single-buffered to fit SBUF (pays for lookahead=2's
        # extra xT buffer); their next-layer DMAs are issued at tile NT-1,
        # after their last reads of the layer, so the WAR waits are already
        # satisfied and the transfers overlap tile NT-1's matmuls.
        wt_pool1 = ctx.enter_context(tc.tile_pool(name="wt1", bufs=1))
        wt_pool = ctx.enter_context(tc.tile_pool(name="wt", bufs=2))
        brep_pool = ctx.enter_context(tc.tile_pool(name="brep", bufs=2))
        xt_pool = ctx.enter_context(tc.tile_pool(name="xt", bufs=lookahead + 1))
        st_pool = ctx.enter_context(tc.tile_pool(name="st", bufs=lookahead + 2))
        acc_psum = ctx.enter_context(tc.tile_pool(name="accp", bufs=2, space="PSUM"))

        eps_t = singles.tile([P, 1], f32)
        nc.vector.memset(eps_t, LN_EPS)

        xres = [xres_pool.tile([P, D], bf16, tag=f"xres{i}", name=f"xres{i}")
                for i in range(NT)]

        def alloc_wts():
            return [
                (wt_pool1 if k <= 1 else wt_pool).tile(
                    [P, D], bf16, tag=f"wt{k}", name=f"wt{k}"
                )
                for k in range(KC)
            ]

        # --- initial loads ---
        # x0 first (prep(0) needs it), then layer-0 W chunks split across
        # BOTH HWDGE queues (SP is idle until the first transpose at ~6us)
        # so the PE's tile-0 k-loop chases two parallel chunk streams; the
        # remaining x tiles trail 1:4 behind on the Act queue.
        wts_cur = alloc_wts()
        brep_cur = brep_pool.tile([P, D], bf16, tag="brep", name="brep")
        # x0 in four 512-col chunks: each bn_stats(g) starts as its chunk
        # lands instead of waiting for the whole tile.
        for g in range(4):
            nc.scalar.dma_start(
                out=xres[0][:, bass.ts(g, 512)], in_=x_v[0][:, bass.ts(g, 512)]
            )
        for k in range(KC):
            eng = nc.sync if k % 2 == 0 else nc.scalar
            eng.dma_start(out=wts_cur[k], in_=wt_v[0, k])
        nc.scalar.dma_start(out=brep_cur, in_=br_d[0])

        def prep(i):
            """LN stats + in-place normalize + transpose for token tile i of
            the current layer; returns the ready-to-contract xT tile."""
            slot = xres[i]
            stats = st_pool.tile([P, 4, 6], f32, tag="bnst", name="stats")
            for g in range(4):
                nc.vector.bn_stats(out=stats[:, g, :], in_=slot[:, bass.ts(g, 512)])
            mv = st_pool.tile([P, 2], f32, tag="mv", name="mv")
            nc.vector.bn_aggr(out=mv, in_=stats)
            rstd = st_pool.tile([P, 1], f32, tag="rstd", name="rstd")
            nc.scalar.activation(
                out=rstd,
                in_=mv[:, 1:2],
                func=mybir.ActivationFunctionType.Sqrt,
                bias=eps_t,
                scale=1.0,
            )
            nc.vector.reciprocal(out=rstd, in_=rstd)
            nc.vector.tensor_scalar(
                out=slot,
                in0=slot,
                scalar1=mv[:, 0:1],
                scalar2=rstd,
                op0=mybir.AluOpType.subtract,
                op1=mybir.AluOpType.mult,
            )
            xT = xt_pool.tile([P, KC, P], bf16, tag="xT", name="xT")
            nc.sync.dma_start_transpose(out=xT, in_=slot)
            return xT

        total = n_steps * NT
        wts_by_step = [wts_cur]
        brep_by_step = [brep_cur]
        from collections import deque

        # prep(0) is emitted BEFORE the x1..15 tile loads so its transpose
        # enters the DMA queue ahead of them: only x0 + the 16 W chunks can
        # delay the first matmul, not the other 7.5MB of x traffic. The
        # remaining x tiles go on the SP queue behind that transpose (SP
        # dispatch is otherwise idle; they're not needed for many tiles).
        q = deque()
        if lookahead >= 1 and total >= 1:
            q.append(prep(0))
        for i in range(1, min(lookahead + 1, NT)):
            nc.sync.dma_start(out=xres[i], in_=x_v[i])
        for j in range(1, min(lookahead, total)):
            q.append(prep(j % NT))

        for j in range(total):
            step, i = divmod(j, NT)
            l = step % n_layers
            # layer 0: dispatch x tiles just-in-time, one per tile period,
            # so they never back up the DMA queue ahead of a transpose.
            if step == 0 and i + lookahead + 1 < NT:
                nc.sync.dma_start(
                    out=xres[i + lookahead + 1], in_=x_v[i + lookahead + 1]
                )
            if j + lookahead < total:
                q.append(prep((j + lookahead) % NT))
            xT = q.popleft()
            wts = wts_by_step[step]
            brep = brep_by_step[step]

            # prefetch layer step+1: chunks 1..15 spread round-robin over the
            # tiles; the single-buffered chunk 0 last (tile NT-1), after its
            # final read of the layer so the WAR wait is already satisfied.
            if step + 1 < n_steps:
                nl = (step + 1) % n_layers
                if i == 0:
                    wts_by_step.append(alloc_wts())
                    bnx = brep_pool.tile([P, D], bf16, tag="brep", name="brep")
                    brep_by_step.append(bnx)
                    nc.scalar.dma_start(out=bnx, in_=br_d[nl])
                for c in range(2, KC):
                    if c % NT == i:
                        nc.scalar.dma_start(
                            out=wts_by_step[step + 1][c], in_=wt_v[nl, c]
                        )
                if i == NT - 1:
                    for c in (0, 1):
                        nc.scalar.dma_start(
                            out=wts_by_step[step + 1][c], in_=wt_v[nl, c]
                        )

            # --- matmuls ---
            accs = [
                acc_psum.tile([P, EW], f32, tag=f"acc{e}", name=f"acc{e}")
                for e in range(EC)
            ]
            slot = xres[i]
            last_tile = j == total - 1
            if not last_tile:
                # k-outer, e-inner: lhsT shared across the 4 e-chunks (LDW
                # elided); epilogue as one block after the tile.
                for k in range(KC):
                    for e in range(EC):
                        nc.tensor.matmul(
                            out=accs[e],
                            lhsT=xT[:, k, :],
                            rhs=wts[k][:, bass.ts(e, EW)],
                            start=(k == 0),
                            stop=(k == KC - 1),
                        )
                for e in range(EC):
                    nc.vector.tensor_add(
                        slot[:, bass.ts(e, EW)], accs[e], brep[:, bass.ts(e, EW)]
                    )
                nc.scalar.activation(
                    out=slot,
                    in_=slot,
                    func=mybir.ActivationFunctionType.Gelu_apprx_tanh,
                )
                if l == n_layers - 1:
                    nc.scalar.dma_start(out=y_v[i], in_=slot)
            else:
                # final tile: e-outer so each PSUM group finishes at 25%/50%/
                # 75%/100% of the tile and its add+gelu+store overlaps the
                # remaining matmuls — shortens the drain tail to one e-chunk.
                # (Costs the LDW elision on this one tile only.)
                for e in range(EC):
                    for k in range(KC):
                        nc.tensor.matmul(
                            out=accs[e],
                            lhsT=xT[:, k, :],
                            rhs=wts[k][:, bass.ts(e, EW)],
                            start=(k == 0),
                            stop=(k == KC - 1),
                        )
                    nc.vector.tensor_add(
                        slot[:, bass.ts(e, EW)], accs[e], brep[:, bass.ts(e, EW)]
                    )
                    nc.scalar.activation(
                        out=slot[:, bass.ts(e, EW)],
                        in_=slot[:, bass.ts(e, EW)],
                        func=mybir.ActivationFunctionType.Gelu_apprx_tanh,
                    )
                    if l == n_layers - 1:
                        nc.scalar.dma_start(
                            out=y_v[i][:, bass.ts(e, EW)],
                            in_=slot[:, bass.ts(e, EW)],
                        )

    _elide_ldweights(nc)
    _split_matmul_waits(nc)


def _elide_ldweights(nc):
    """The tile finalize pass splits 16-bit matmuls into explicit
    InstLdweights + InstMatmult pairs, one load per matmul. Consecutive
    matmuls here share the stationary operand (lhsT reused across the 4
    e-chunks), so 3 of every 4 loads are redundant — the PE array already
    holds the weights. Drop them (keeping any sync as an EventSemaphore);
    saves 128 load-rows per 512 matmul rows (~20% PE time)."""
    import concourse.mybir as mybir

    n = 0
    for fn in nc.m.functions:
        for bb in fn.blocks:
            insts = bb.instructions
            last_key = None
            i = 0
            while i < len(insts):
                inst = insts[i]
                if getattr(inst, "engine", None) == mybir.EngineType.PE:
                    tn = type(inst).__name__
                    if tn == "InstLdweights":
                        ap = inst.ins[0]
                        key = (
                            str(getattr(ap, "memref", None)),
                            getattr(ap, "offset", None),
                            str(getattr(ap, "ap", None)),
                            str(getattr(ap, "dtype", None)),
                            bool(inst.is_transpose),
                            str(inst.perf_mode),
                        )
                        if key == last_key:
                            si = inst.sync_info
                            if si and (si.on_wait or si.on_update):
                                ev = mybir.InstEventSemaphore(
                                    name=f"{inst.name}-ldwe",
                                    engine=inst.engine,
                                    sync_info=si,
                                )
                                nc.register_instruction(ev, overwrite=True)
                                insts[i] = ev
                                i += 1
                            else:
                                del insts[i]
                            n += 1
                            continue
                        last_key = key
                    elif tn in ("InstMatmult", "InstEventSemaphore"):
                        pass
                    else:
                        last_key = None
                i += 1
    return n


def _split_matmul_waits(nc):
    """Walrus encodes fp32/fp32r/transpose matmuls as self-loading LW-struct
    instructions, which accept at most ONE sync-wait command. Tile's wait
    assignment can attach several. Hoist all but one wait of each matmult onto
    standalone EventSemaphore (sequencer) instructions inserted right before
    it on the same engine — semantically identical, codegen-legal."""
    import concourse.mybir as mybir

    skip = ("InstEventSemaphore",)
    n_split = 0
    for fn in nc.m.functions:
        for bb in fn.blocks:
            insts = bb.instructions
            i = 0
            while i < len(insts):
                inst = insts[i]
                if type(inst).__name__ not in skip:
                    si = inst.sync_info
                    waits = list(si.on_wait) if (si and si.on_wait) else []
                    if len(waits) > 1:
                        for j, w in enumerate(waits[:-1]):
                            ev = mybir.InstEventSemaphore(
                                name=f"{inst.name}-hw{j}",
                                engine=inst.engine,
                                sync_info=mybir.SyncInfo(
                                    on_wait=[w], on_update=[]
                                ),
                            )
                            nc.register_instruction(ev, overwrite=True)
                            insts.insert(i, ev)
                            i += 1
                        si.on_wait = [waits[-1]]
                        n_split += 1
                i += 1
    return n_split


_CACHE = {}


def _get_nc():
    if "nc" not in _CACHE:
        import concourse.bass as bass

        nc = bass.Bass("TRN2", target_bir_lowering=False)
        build(nc)
        _CACHE["nc"] = nc
    return _CACHE["nc"]


def _prep_host(x, W, b, ln_w, ln_b):
    """Fold LN affine into weights; pre-transpose W to [L, D_in, D_out];
    cast everything to bf16 and build the replicated bias input."""
    import ml_dtypes

    bf = ml_dtypes.bfloat16
    x = np.asarray(x, dtype=np.float32)
    W = np.asarray(W, dtype=np.float32)
    b = np.asarray(b, dtype=np.float32)
    ln_w = np.asarray(ln_w, dtype=np.float32)
    ln_b = np.asarray(ln_b, dtype=np.float32)

    Wf = W * ln_w[:, None, :]  # scale columns (input dim)
    bfold = b + np.einsum("led,ld->le", W, ln_b)
    WT = np.ascontiguousarray(Wf.transpose(0, 2, 1)).astype(bf)  # [L, Din, E]
    brep = np.ascontiguousarray(
        np.broadcast_to(bfold[:, None, :], (bfold.shape[0], P, D))
    ).astype(bf)
    xb = np.ascontiguousarray(x).astype(bf)
    return xb, WT, brep


def make_in_maps(inputs):
    xb, WT, brep = _prep_host(**inputs)
    return [{"x": xb[i], "wt": WT, "brep": brep} for i in range(B)]


def run(x, W, b, ln_w, ln_b, trace=False):
    from concourse import bass_utils

    in_maps = make_in_maps(dict(x=x, W=W, b=b, ln_w=ln_w, ln_b=ln_b))
    nc = _get_nc()
    res = bass_utils.run_bass_kernel_spmd(
        nc, in_maps, core_ids=list(range(B)), trace=trace
    )
    out = np.stack(
        [np.asarray(res.results[i]["y"], dtype=np.float32) for i in range(B)]
    )
    return out.reshape(B, S, D), res


def kernel(x, W, b, ln_w, ln_b):
    out, _ = run(x, W, b, ln_w, ln_b)
    return out


# revision 20
# speedup vs baseline: 1.0830x; 1.0830x over previous
"""Trainium2 Bass kernel for nn_AttnMLP: 4x (LayerNorm -> Linear(2048,2048) -> tanh-GELU).

Sharding: data-parallel, batch dim (8 batch elements) across 8 NeuronCores.

v2 design (vs the fp32r baseline):
  - bf16 activations+weights (matmul rate is 1 cyc/row for both fp32r and
    bf16 on TRN2, but bf16 halves DMA traffic and SBUF footprint).
  - Activations SBUF-resident across all 4 layers: 16 slots of [128, 2048]
    bf16, written in place by the epilogue (no DRAM round-trips).
  - x-transposes moved off the PE onto the DMA XBAR: one dma_start_transpose
    per token tile (SBUF->SBUF, bf16) produces xT [128, 16, 128] directly.
    PE stream is pure matmul (~874us/pass floor at 2.4GHz).
  - Weights streamed per layer in bf16, double-buffered (prefetched one
    layer ahead, one chunk per token tile, on the Act HWDGE queue; the
    latency-critical transposes have the SP HWDGE queue to themselves).
  - Bias broadcast [128, 2048] precomputed on host (brep input).
  - LN affine (ln_w, ln_b) folded into W and b on the host:
      W' = W * ln_w[None, :],  b' = b + W @ ln_b
  - Output written bf16, upcast to fp32 on host.
  - The tile finalizer splits bf16 matmuls into Ldweights+Matmult pairs;
    _elide_ldweights drops the 3/4 redundant loads (lhsT shared across the
    4 e-chunk matmuls), leaving the structural 1-per-(tile,k) load.
  - Final tile runs e-outer (each PSUM group completes early; its
    add+gelu+store overlaps the remaining matmuls) to shorten the drain
    tail; costs the LDW elision on that one tile only.
  - lookahead=2 (clamped to NT-1): prep runs two tiles ahead so real-HW
    jitter in the DVE/XBAR prep chain cannot stall the PE.

Measured (8-core, axon, min-statistic repeat-slope): ~880-930 us
steady-state per 4-layer pass (vs ~1170-1240 us for the fp32r
DRAM-round-trip baseline, same estimator) — at the structural floor:
872 us matmul rows (1 cyc/row @ 2.4 GHz) + ~56 us LDWEIGHTS. Max rel err
1.06e-2 vs the fp32 reference (bf16 quantization, ~0.26%/layer).

Per-core per-tile pipeline (software-pipelined `lookahead` tiles ahead):
  prep(i):  DVE bn_stats/bn_aggr -> rstd (ScalarE sqrt, DVE recip);
            DVE in-place normalize of resident slot; SP dma_start_transpose
            slot -> xT[128, 16, 128]
  matmul(i): for k in 16: lhsT=xT[:,k,:] (stationary, LDW elided over e);
             for e in 4: PSUM[e] += xT_k.T @ W_k[:, 512e] (bf16, 512 rows)
  epilogue(i): DVE tensor_add (PSUM + brep -> slot, bf16), ScalarE
             Gelu_apprx_tanh in place; last layer: DMA slot -> y.
"""

import sys

sys.path.insert(0, "/opt/trn_rl_repo")

import numpy as np

N_LAYERS = 4
D = 2048  # embedding dim
B = 8  # batch (one element per core)
S = 2048  # sequence length
T = S  # tokens per core
P = 128  # partitions
KC = D // P  # 16 contraction chunks
EC = 4  # output-feature chunks (PSUM banks per tile)
EW = D // EC  # 512 output features per chunk
LN_EPS = 1e-5


def build(nc, T_tokens=T, n_layers=N_LAYERS, repeat=1, lookahead=2,
          transpose_mode="dma"):
    """Emit the kernel IR into `nc`.

    repeat > 1 re-runs the whole n_layers stack that many times (reusing the
    same weights) — a timing-only amplifier for slope measurements."""
    import concourse.bass as bass
    import concourse.mybir as mybir
    import concourse.tile as tile
    from contextlib import ExitStack

    f32 = mybir.dt.float32
    bf16 = mybir.dt.bfloat16
    NT = T_tokens // P  # token tiles
    # prep(j+lookahead) reads slot (j+lookahead) % NT, which epilogue(j)
    # has not yet written when the slot index wraps -> lookahead < NT.
    lookahead = min(lookahead, NT - 1) if NT > 1 else 0
    n_steps = n_layers * repeat

    x_d = nc.dram_tensor("x", [T_tokens, D], bf16, kind="ExternalInput")
    wt_d = nc.dram_tensor("wt", [n_layers, D, D], bf16, kind="ExternalInput")
    br_d = nc.dram_tensor("brep", [n_layers, P, D], bf16, kind="ExternalInput")
    y_d = nc.dram_tensor("y", [T_tokens, D], bf16, kind="ExternalOutput")

    x_v = x_d.rearrange("(n p) d -> n p d", p=P)
    y_v = y_d.rearrange("(n p) d -> n p d", p=P)
    wt_v = wt_d.rearrange("l (kc p) e -> l kc p e", p=P)  # [L, 16, 128, 2048]

    with tile.TileContext(nc) as tc, ExitStack() as ctx:
        singles = ctx.enter_context(tc.tile_pool(name="singles", bufs=1))
        xres_pool = ctx.enter_context(tc.tile_pool(name="xres", bufs=1))
        # chunks 0-1 single-buffered to fit SBUF (pays for lookahead=2's
        # extra xT buffer); their next-layer DMAs are issued at tile NT-1,
        # after their last reads of the layer, so the WAR waits are already
        # satisfied and the transfers overlap tile NT-1's matmuls.
        wt_pool1 = ctx.enter_context(tc.tile_pool(name="wt1", bufs=1))
        wt_pool = ctx.enter_context(tc.tile_pool(name="wt", bufs=2))
        brep_pool = ctx.enter_context(tc.tile_pool(name="brep", bufs=2))
        xt_pool = ctx.enter_context(tc.tile_pool(name="xt", bufs=lookahead + 1))
        st_pool = ctx.enter_context(tc.tile_pool(name="st", bufs=lookahead + 2))
        acc_psum = ctx.enter_context(tc.tile_pool(name="accp", bufs=2, space="PSUM"))

        eps_t = singles.tile([P, 1], f32)
        nc.vector.memset(eps_t, LN_EPS)

        xres = [xres_pool.tile([P, D], bf16, tag=f"xres{i}", name=f"xres{i}")
                for i in range(NT)]

        def alloc_wts():
            return [
                (wt_pool1 if k <= 1 else wt_pool).tile(
                    [P, D], bf16, tag=f"wt{k}", name=f"wt{k}"
                )
                for k in range(KC)
            ]

        # --- initial loads ---
        # x0 first (prep(0) needs it), then layer-0 W chunks split across
        # BOTH HWDGE queues (SP is idle until the first transpose at ~6us)
        # so the PE's tile-0 k-loop chases two parallel chunk streams; the
        # remaining x tiles trail 1:4 behind on the Act queue.
        wts_cur = alloc_wts()
        brep_cur = brep_pool.tile([P, D], bf16, tag="brep", name="brep")
        # x0 in four 512-col chunks: each bn_stats(g) starts as its chunk
        # lands instead of waiting for the whole tile.
        for g in range(4):
            nc.scalar.dma_start(
                out=xres[0][:, bass.ts(g, 512)], in_=x_v[0][:, bass.ts(g, 512)]
            )
        for k in range(0, KC, 2):
            nc.sync.dma_start(out=wts_cur[k], in_=wt_v[0, k])
        nc.scalar.dma_start(out=wts_cur[1], in_=wt_v[0, 1])

        def prep(i):
            """LN stats + in-place normalize + transpose for token tile i of
            the current layer; returns the ready-to-contract xT tile."""
            slot = xres[i]
            stats = st_pool.tile([P, 4, 6], f32, tag="bnst", name="stats")
            for g in range(4):
                nc.vector.bn_stats(out=stats[:, g, :], in_=slot[:, bass.ts(g, 512)])
            mv = st_pool.tile([P, 2], f32, tag="mv", name="mv")
            nc.vector.bn_aggr(out=mv, in_=stats)
            rstd = st_pool.tile([P, 1], f32, tag="rstd", name="rstd")
            nc.scalar.activation(
                out=rstd,
                in_=mv[:, 1:2],
                func=mybir.ActivationFunctionType.Sqrt,
                bias=eps_t,
                scale=1.0,
            )
            nc.vector.reciprocal(out=rstd, in_=rstd)
            nc.vector.tensor_scalar(
                out=slot,
                in0=slot,
                scalar1=mv[:, 0:1],
                scalar2=rstd,
                op0=mybir.AluOpType.subtract,
                op1=mybir.AluOpType.mult,
            )
            xT = xt_pool.tile([P, KC, P], bf16, tag="xT", name="xT")
            nc.sync.dma_start_transpose(out=xT, in_=slot)
            return xT

        total = n_steps * NT
        wts_by_step = [wts_cur]
        brep_by_step = [brep_cur]
        from collections import deque

        # prep(0) is emitted BEFORE the x1..15 tile loads so its transpose
        # enters the DMA queue ahead of them: only x0 + the 16 W chunks can
        # delay the first matmul, not the other 7.5MB of x traffic. The
        # remaining x tiles go on the SP queue behind that transpose (SP
        # dispatch is otherwise idle; they're not needed for many tiles).
        # prep(0) is emitted before the odd W chunks: on queue order, its SP
        # transpose then precedes only the 8 even chunks + k1, and the odd
        # chunks k3..15 trail behind it — they stream in just ahead of the
        # PE's tile-0 k-loop instead of delaying its start.
        q = deque()
        if lookahead >= 1 and total >= 1:
            q.append(prep(0))
        for k in range(3, KC, 2):
            nc.scalar.dma_start(out=wts_cur[k], in_=wt_v[0, k])
        nc.scalar.dma_start(out=brep_cur, in_=br_d[0])
        for i in range(1, min(lookahead + 1, NT)):
            nc.sync.dma_start(out=xres[i], in_=x_v[i])
        for j in range(1, min(lookahead, total)):
            q.append(prep(j % NT))

        for j in range(total):
            step, i = divmod(j, NT)
            l = step % n_layers
            # layer 0: dispatch x tiles just-in-time, one per tile period,
            # so they never back up the DMA queue ahead of a transpose.
            if step == 0 and i + lookahead + 1 < NT:
                nc.sync.dma_start(
                    out=xres[i + lookahead + 1], in_=x_v[i + lookahead + 1]
                )
            if j + lookahead < total:
                q.append(prep((j + lookahead) % NT))
            xT = q.popleft()
            wts = wts_by_step[step]
            brep = brep_by_step[step]

            # prefetch layer step+1: chunks 1..15 spread round-robin over the
            # tiles; the single-buffered chunk 0 last (tile NT-1), after its
            # final read of the layer so the WAR wait is already satisfied.
            if step + 1 < n_steps:
                nl = (step + 1) % n_layers
                if i == 0:
                    wts_by_step.append(alloc_wts())
                    bnx = brep_pool.tile([P, D], bf16, tag="brep", name="brep")
                    brep_by_step.append(bnx)
                    nc.scalar.dma_start(out=bnx, in_=br_d[nl])
                for c in range(2, KC):
                    if c % NT == i:
                        nc.scalar.dma_start(
                            out=wts_by_step[step + 1][c], in_=wt_v[nl, c]
                        )
                if i == NT - 1:
                    for c in (0, 1):
                        nc.scalar.dma_start(
                            out=wts_by_step[step + 1][c], in_=wt_v[nl, c]
                        )

            # --- matmuls ---
            accs = [
                acc_psum.tile([P, EW], f32, tag=f"acc{e}", name=f"acc{e}")
                for e in range(EC)
            ]
            slot = xres[i]
            last_tile = j == total - 1
            if not last_tile:
                # k-outer, e-inner: lhsT shared across the 4 e-chunks (LDW
                # elided); epilogue as one block after the tile.
                for k in range(KC):
                    for e in range(EC):
                        nc.tensor.matmul(
                            out=accs[e],
                            lhsT=xT[:, k, :],
                            rhs=wts[k][:, bass.ts(e, EW)],
                            start=(k == 0),
                            stop=(k == KC - 1),
                        )
                for e in range(EC):
                    nc.vector.tensor_add(
                        slot[:, bass.ts(e, EW)], accs[e], brep[:, bass.ts(e, EW)]
                    )
                nc.scalar.activation(
                    out=slot,
                    in_=slot,
                    func=mybir.ActivationFunctionType.Gelu_apprx_tanh,
                )
                if l == n_layers - 1:
                    nc.scalar.dma_start(out=y_v[i], in_=slot)
            else:
                # final tile: e-outer so each PSUM group finishes at 25%/50%/
                # 75%/100% of the tile and its add+gelu+store overlaps the
                # remaining matmuls — shortens the drain tail to one e-chunk.
                # (Costs the LDW elision on this one tile only.)
                for e in range(EC):
                    for k in range(KC):
                        nc.tensor.matmul(
                            out=accs[e],
                            lhsT=xT[:, k, :],
                            rhs=wts[k][:, bass.ts(e, EW)],
                            start=(k == 0),
                            stop=(k == KC - 1),
                        )
                    nc.vector.tensor_add(
                        slot[:, bass.ts(e, EW)], accs[e], brep[:, bass.ts(e, EW)]
                    )
                    nc.scalar.activation(
                        out=slot[:, bass.ts(e, EW)],
                        in_=slot[:, bass.ts(e, EW)],
                        func=mybir.ActivationFunctionType.Gelu_apprx_tanh,
                    )
                    if l == n_layers - 1:
                        nc.scalar.dma_start(
                            out=y_v[i][:, bass.ts(e, EW)],
                            in_=slot[:, bass.ts(e, EW)],
                        )

    _elide_ldweights(nc)
    _split_matmul_waits(nc)


def _elide_ldweights(nc):
    """The tile finalize pass splits 16-bit matmuls into explicit
    InstLdweights + InstMatmult pairs, one load per matmul. Consecutive
    matmuls here share the stationary operand (lhsT reused across the 4
    e-chunks), so 3 of every 4 loads are redundant — the PE array already
    holds the weights. Drop them (keeping any sync as an EventSemaphore);
    saves 128 load-rows per 512 matmul rows (~20% PE time)."""
    import concourse.mybir as mybir

    n = 0
    for fn in nc.m.functions:
        for bb in fn.blocks:
            insts = bb.instructions
            last_key = None
            i = 0
            while i < len(insts):
                inst = insts[i]
                if getattr(inst, "engine", None) == mybir.EngineType.PE:
                    tn = type(inst).__name__
                    if tn == "InstLdweights":
                        ap = inst.ins[0]
                        key = (
                            str(getattr(ap, "memref", None)),
                            getattr(ap, "offset", None),
                            str(getattr(ap, "ap", None)),
                            str(getattr(ap, "dtype", None)),
                            bool(inst.is_transpose),
                            str(inst.perf_mode),
                        )
                        if key == last_key:
                            si = inst.sync_info
                            if si and (si.on_wait or si.on_update):
                                ev = mybir.InstEventSemaphore(
                                    name=f"{inst.name}-ldwe",
                                    engine=inst.engine,
                                    sync_info=si,
                                )
                                nc.register_instruction(ev, overwrite=True)
                                insts[i] = ev
                                i += 1
                            else:
                                del insts[i]
                            n += 1
                            continue
                        last_key = key
                    elif tn in ("InstMatmult", "InstEventSemaphore"):
                        pass
                    else:
                        last_key = None
                i += 1
    return n


def _split_matmul_waits(nc):
    """Walrus encodes fp32/fp32r/transpose matmuls as self-loading LW-struct
    instructions, which accept at most ONE sync-wait command. Tile's wait
    assignment can attach several. Hoist all but one wait of each matmult onto
    standalone EventSemaphore (sequencer) instructions inserted right before
    it on the same engine — semantically identical, codegen-legal."""
    import concourse.mybir as mybir

    skip = ("InstEventSemaphore",)
    n_split = 0
    for fn in nc.m.functions:
        for bb in fn.blocks:
            insts = bb.instructions
            i = 0
            while i < len(insts):
                inst = insts[i]
                if type(inst).__name__ not in skip:
                    si = inst.sync_info
                    waits = list(si.on_wait) if (si and si.on_wait) else []
                    if len(waits) > 1:
                        for j, w in enumerate(waits[:-1]):
                            ev = mybir.InstEventSemaphore(
                                name=f"{inst.name}-hw{j}",
                                engine=inst.engine,
                                sync_info=mybir.SyncInfo(
                                    on_wait=[w], on_update=[]
                                ),
                            )
                            nc.register_instruction(ev, overwrite=True)
                            insts.insert(i, ev)
                            i += 1
                        si.on_wait = [waits[-1]]
                        n_split += 1
                i += 1
    return n_split


_CACHE = {}


def _get_nc():
    if "nc" not in _CACHE:
        import concourse.bass as bass

        nc = bass.Bass("TRN2", target_bir_lowering=False)
        build(nc)
        _CACHE["nc"] = nc
    return _CACHE["nc"]


def _prep_host(x, W, b, ln_w, ln_b):
    """Fold LN affine into weights; pre-transpose W to [L, D_in, D_out];
    cast everything to bf16 and build the replicated bias input."""
    import ml_dtypes

    bf = ml_dtypes.bfloat16
    x = np.asarray(x, dtype=np.float32)
    W = np.asarray(W, dtype=np.float32)
    b = np.asarray(b, dtype=np.float32)
    ln_w = np.asarray(ln_w, dtype=np.float32)
    ln_b = np.asarray(ln_b, dtype=np.float32)

    Wf = W * ln_w[:, None, :]  # scale columns (input dim)
    bfold = b + np.einsum("led,ld->le", W, ln_b)
    WT = np.ascontiguousarray(Wf.transpose(0, 2, 1)).astype(bf)  # [L, Din, E]
    brep = np.ascontiguousarray(
        np.broadcast_to(bfold[:, None, :], (bfold.shape[0], P, D))
    ).astype(bf)
    xb = np.ascontiguousarray(x).astype(bf)
    return xb, WT, brep


def make_in_maps(inputs):
    xb, WT, brep = _prep_host(**inputs)
    return [{"x": xb[i], "wt": WT, "brep": brep} for i in range(B)]


def run(x, W, b, ln_w, ln_b, trace=False):
    from concourse import bass_utils

    in_maps = make_in_maps(dict(x=x, W=W, b=b, ln_w=ln_w, ln_b=ln_b))
    nc = _get_nc()
    res = bass_utils.run_bass_kernel_spmd(
        nc, in_maps, core_ids=list(range(B)), trace=trace
    )
    out = np.stack(
        [np.asarray(res.results[i]["y"], dtype=np.float32) for i in range(B)]
    )
    return out.reshape(B, S, D), res


def kernel(x, W, b, ln_w, ln_b):
    out, _ = run(x, W, b, ln_w, ln_b)
    return out


# revision 25
# speedup vs baseline: 1.2382x; 1.1433x over previous
"""Trainium2 Bass kernel for nn_AttnMLP: 4x (LayerNorm -> Linear(2048,2048) -> tanh-GELU).

Sharding: data-parallel, batch dim (8 batch elements) across 8 NeuronCores.

v2 design (vs the fp32r baseline):
  - bf16 activations+weights (matmul rate is 1 cyc/row for both fp32r and
    bf16 on TRN2, but bf16 halves DMA traffic and SBUF footprint).
  - Activations SBUF-resident across all 4 layers: 16 slots of [128, 2048]
    bf16, written in place by the epilogue (no DRAM round-trips).
  - x-transposes moved off the PE onto the DMA XBAR: one dma_start_transpose
    per token tile (SBUF->SBUF, bf16) produces xT [128, 16, 128] directly.
    PE stream is pure matmul (~874us/pass floor at 2.4GHz).
  - Weights streamed per layer in bf16, double-buffered (prefetched one
    layer ahead, one chunk per token tile, on the Act HWDGE queue; the
    latency-critical transposes have the SP HWDGE queue to themselves).
  - Bias broadcast [128, 2048] precomputed on host (brep input).
  - LN affine (ln_w, ln_b) folded into W and b on the host:
      W' = W * ln_w[None, :],  b' = b + W @ ln_b
  - Output written bf16, upcast to fp32 on host.
  - The tile finalizer splits bf16 matmuls into Ldweights+Matmult pairs;
    _elide_ldweights drops the 3/4 redundant loads (lhsT shared across the
    4 e-chunk matmuls), leaving the structural 1-per-(tile,k) load.
  - Final tile runs e-outer (each PSUM group completes early; its
    add+gelu+store overlaps the remaining matmuls) to shorten the drain
    tail; costs the LDW elision on that one tile only.
  - lookahead=2 (clamped to NT-1): prep runs two tiles ahead so real-HW
    jitter in the DVE/XBAR prep chain cannot stall the PE.

Measured (8-core, axon, min-statistic repeat-slope): ~880-930 us
steady-state per 4-layer pass (vs ~1170-1240 us for the fp32r
DRAM-round-trip baseline, same estimator) — at the structural floor:
872 us matmul rows (1 cyc/row @ 2.4 GHz) + ~56 us LDWEIGHTS. Max rel err
1.06e-2 vs the fp32 reference (bf16 quantization, ~0.26%/layer).

Per-core per-tile pipeline (software-pipelined `lookahead` tiles ahead):
  prep(i):  DVE bn_stats/bn_aggr -> rstd (ScalarE sqrt, DVE recip);
            DVE in-place normalize of resident slot; SP dma_start_transpose
            slot -> xT[128, 16, 128]
  matmul(i): for k in 16: lhsT=xT[:,k,:] (stationary, LDW elided over e);
             for e in 4: PSUM[e] += xT_k.T @ W_k[:, 512e] (bf16, 512 rows)
  epilogue(i): DVE tensor_add (PSUM + brep -> slot, bf16), ScalarE
             Gelu_apprx_tanh in place; last layer: DMA slot -> y.
"""

import sys

sys.path.insert(0, "/opt/trn_rl_repo")

import numpy as np

N_LAYERS = 4
D = 2048  # embedding dim
B = 8  # batch (one element per core)
S = 2048  # sequence length
T = S  # tokens per core
P = 128  # partitions
KC = D // P  # 16 contraction chunks
EC = 4  # output-feature chunks (PSUM banks per tile)
EW = D // EC  # 512 output features per chunk
LN_EPS = 1e-5


def build(nc, T_tokens=T, n_layers=N_LAYERS, repeat=1, lookahead=2,
          transpose_mode="dma"):
    """Emit the kernel IR into `nc`.

    repeat > 1 re-runs the whole n_layers stack that many times (reusing the
    same weights) — a timing-only amplifier for slope measurements."""
    import concourse.bass as bass
    import concourse.mybir as mybir
    import concourse.tile as tile
    from contextlib import ExitStack

    f32 = mybir.dt.float32
    bf16 = mybir.dt.bfloat16
    NT = T_tokens // P  # token tiles
    # prep(j+lookahead) reads slot (j+lookahead) % NT, which epilogue(j)
    # has not yet written when the slot index wraps -> lookahead < NT.
    lookahead = min(lookahead, NT - 1) if NT > 1 else 0
    n_steps = n_layers * repeat

    x_d = nc.dram_tensor("x", [T_tokens, D], bf16, kind="ExternalInput")
    wt_d = nc.dram_tensor("wt", [n_layers, D, D], bf16, kind="ExternalInput")
    br_d = nc.dram_tensor("brep", [n_layers, P, D], bf16, kind="ExternalInput")
    y_d = nc.dram_tensor("y", [T_tokens, D], bf16, kind="ExternalOutput")

    x_v = x_d.rearrange("(n p) d -> n p d", p=P)
    y_v = y_d.rearrange("(n p) d -> n p d", p=P)
    wt_v = wt_d.rearrange("l (kc p) e -> l kc p e", p=P)  # [L, 16, 128, 2048]

    with tile.TileContext(nc) as tc, ExitStack() as ctx:
        singles = ctx.enter_context(tc.tile_pool(name="singles", bufs=1))
        xres_pool = ctx.enter_context(tc.tile_pool(name="xres", bufs=1))
        # chunks 0-1 single-buffered to fit SBUF (pays for lookahead=2's
        # extra xT buffer); their next-layer DMAs are issued at tile NT-1,
        # after their last reads of the layer, so the WAR waits are already
        # satisfied and the transfers overlap tile NT-1's matmuls.
        wt_pool1 = ctx.enter_context(tc.tile_pool(name="wt1", bufs=1))
        wt_pool = ctx.enter_context(tc.tile_pool(name="wt", bufs=2))
        brep_pool = ctx.enter_context(tc.tile_pool(name="brep", bufs=2))
        xt_pool = ctx.enter_context(tc.tile_pool(name="xt", bufs=lookahead + 1))
        st_pool = ctx.enter_context(tc.tile_pool(name="st", bufs=lookahead + 2))
        acc_psum = ctx.enter_context(tc.tile_pool(name="accp", bufs=2, space="PSUM"))

        eps_t = singles.tile([P, 1], f32)
        nc.vector.memset(eps_t, LN_EPS)

        xres = [xres_pool.tile([P, D], bf16, tag=f"xres{i}", name=f"xres{i}")
                for i in range(NT)]

        def alloc_wts():
            return [
                (wt_pool1 if k <= 1 else wt_pool).tile(
                    [P, D], bf16, tag=f"wt{k}", name=f"wt{k}"
                )
                for k in range(KC)
            ]

        # --- initial loads ---
        # x0 first (prep(0) needs it), then layer-0 W chunks split across
        # BOTH HWDGE queues (SP is idle until the first transpose at ~6us)
        # so the PE's tile-0 k-loop chases two parallel chunk streams; the
        # remaining x tiles trail 1:4 behind on the Act queue.
        wts_cur = alloc_wts()
        brep_cur = brep_pool.tile([P, D], bf16, tag="brep", name="brep")
        # x0 in four 512-col chunks: each bn_stats(g) starts as its chunk
        # lands instead of waiting for the whole tile.
        for g in range(4):
            nc.scalar.dma_start(
                out=xres[0][:, bass.ts(g, 512)], in_=x_v[0][:, bass.ts(g, 512)]
            )
        for k in range(0, KC, 2):
            nc.sync.dma_start(out=wts_cur[k], in_=wt_v[0, k])
        nc.scalar.dma_start(out=wts_cur[1], in_=wt_v[0, 1])

        def prep(i):
            """LN stats + in-place normalize + transpose for token tile i of
            the current layer; returns the ready-to-contract xT tile."""
            slot = xres[i]
            stats = st_pool.tile([P, 4, 6], f32, tag="bnst", name="stats")
            for g in range(4):
                nc.vector.bn_stats(out=stats[:, g, :], in_=slot[:, bass.ts(g, 512)])
            mv = st_pool.tile([P, 2], f32, tag="mv", name="mv")
            nc.vector.bn_aggr(out=mv, in_=stats)
            rstd = st_pool.tile([P, 1], f32, tag="rstd", name="rstd")
            nc.scalar.activation(
                out=rstd,
                in_=mv[:, 1:2],
                func=mybir.ActivationFunctionType.Sqrt,
                bias=eps_t,
                scale=1.0,
            )
            nc.vector.reciprocal(out=rstd, in_=rstd)
            nc.vector.tensor_scalar(
                out=slot,
                in0=slot,
                scalar1=mv[:, 0:1],
                scalar2=rstd,
                op0=mybir.AluOpType.subtract,
                op1=mybir.AluOpType.mult,
            )
            xT = xt_pool.tile([P, KC, P], bf16, tag="xT", name="xT")
            nc.sync.dma_start_transpose(out=xT, in_=slot)
            return xT

        total = n_steps * NT
        wts_by_step = [wts_cur]
        brep_by_step = [brep_cur]
        from collections import deque

        # prep(0) is emitted BEFORE the x1..15 tile loads so its transpose
        # enters the DMA queue ahead of them: only x0 + the 16 W chunks can
        # delay the first matmul, not the other 7.5MB of x traffic. The
        # remaining x tiles go on the SP queue behind that transpose (SP
        # dispatch is otherwise idle; they're not needed for many tiles).
        # prep(0) is emitted before the odd W chunks: on queue order, its SP
        # transpose then precedes only the 8 even chunks + k1, and the odd
        # chunks k3..15 trail behind it — they stream in just ahead of the
        # PE's tile-0 k-loop instead of delaying its start.
        q = deque()
        if lookahead >= 1 and total >= 1:
            q.append(prep(0))
        for k in range(3, KC, 2):
            nc.scalar.dma_start(out=wts_cur[k], in_=wt_v[0, k])
        nc.scalar.dma_start(out=brep_cur, in_=br_d[0])
        for i in range(1, min(lookahead + 1, NT)):
            nc.sync.dma_start(out=xres[i], in_=x_v[i])
        for j in range(1, min(lookahead, total)):
            q.append(prep(j % NT))

        for j in range(total):
            step, i = divmod(j, NT)
            l = step % n_layers
            # layer 0: dispatch x tiles just-in-time, one per tile period,
            # so they never back up the DMA queue ahead of a transpose.
            if step == 0 and i + lookahead + 1 < NT:
                nc.sync.dma_start(
                    out=xres[i + lookahead + 1], in_=x_v[i + lookahead + 1]
                )
            if j + lookahead < total:
                q.append(prep((j + lookahead) % NT))
            xT = q.popleft()
            wts = wts_by_step[step]
            brep = brep_by_step[step]

            # prefetch layer step+1: chunks 1..15 spread round-robin over the
            # tiles; the single-buffered chunk 0 last (tile NT-1), after its
            # final read of the layer so the WAR wait is already satisfied.
            if step + 1 < n_steps:
                nl = (step + 1) % n_layers
                if i == 0:
                    wts_by_step.append(alloc_wts())
                    bnx = brep_pool.tile([P, D], bf16, tag="brep", name="brep")
                    brep_by_step.append(bnx)
                    nc.scalar.dma_start(out=bnx, in_=br_d[nl])
                for c in range(2, KC):
                    if c % NT == i:
                        nc.scalar.dma_start(
                            out=wts_by_step[step + 1][c], in_=wt_v[nl, c]
                        )
                if i == NT - 1:
                    for c in (0, 1):
                        nc.scalar.dma_start(
                            out=wts_by_step[step + 1][c], in_=wt_v[nl, c]
                        )

            # --- matmuls ---
            accs = [
                acc_psum.tile([P, EW], f32, tag=f"acc{e}", name=f"acc{e}")
                for e in range(EC)
            ]
            slot = xres[i]
            last_tile = j == total - 1
            if not last_tile:
                # k-outer, e-inner: lhsT shared across the 4 e-chunks (LDW
                # elided); epilogue as one block after the tile.
                for k in range(KC):
                    for e in range(EC):
                        nc.tensor.matmul(
                            out=accs[e],
                            lhsT=xT[:, k, :],
                            rhs=wts[k][:, bass.ts(e, EW)],
                            start=(k == 0),
                            stop=(k == KC - 1),
                        )
                for e in range(EC):
                    nc.vector.tensor_add(
                        slot[:, bass.ts(e, EW)], accs[e], brep[:, bass.ts(e, EW)]
                    )
                nc.scalar.activation(
                    out=slot,
                    in_=slot,
                    func=mybir.ActivationFunctionType.Gelu_apprx_tanh,
                )
                if l == n_layers - 1:
                    nc.scalar.dma_start(out=y_v[i], in_=slot)
            else:
                # final tile: e-outer so each PSUM group finishes at 25%/50%/
                # 75%/100% of the tile and its add+gelu+store overlaps the
                # remaining matmuls — shortens the drain tail to one e-chunk.
                # (Costs the LDW elision on this one tile only.)
                for e in range(EC):
                    for k in range(KC):
                        nc.tensor.matmul(
                            out=accs[e],
                            lhsT=xT[:, k, :],
                            rhs=wts[k][:, bass.ts(e, EW)],
                            start=(k == 0),
                            stop=(k == KC - 1),
                        )
                    nc.vector.tensor_add(
                        slot[:, bass.ts(e, EW)], accs[e], brep[:, bass.ts(e, EW)]
                    )
                    nc.scalar.activation(
                        out=slot[:, bass.ts(e, EW)],
                        in_=slot[:, bass.ts(e, EW)],
                        func=mybir.ActivationFunctionType.Gelu_apprx_tanh,
                    )
                    if l == n_layers - 1:
                        nc.scalar.dma_start(
                            out=y_v[i][:, bass.ts(e, EW)],
                            in_=slot[:, bass.ts(e, EW)],
                        )

    _elide_ldweights(nc)
    _split_matmul_waits(nc)


def _elide_ldweights(nc):
    """The tile finalize pass splits 16-bit matmuls into explicit
    InstLdweights + InstMatmult pairs, one load per matmul. Consecutive
    matmuls here share the stationary operand (lhsT reused across the 4
    e-chunks), so 3 of every 4 loads are redundant — the PE array already
    holds the weights. Drop them (keeping any sync as an EventSemaphore);
    saves 128 load-rows per 512 matmul rows (~20% PE time)."""
    import concourse.mybir as mybir

    n = 0
    for fn in nc.m.functions:
        for bb in fn.blocks:
            insts = bb.instructions
            last_key = None
            i = 0
            while i < len(insts):
                inst = insts[i]
                if getattr(inst, "engine", None) == mybir.EngineType.PE:
                    tn = type(inst).__name__
                    if tn == "InstLdweights":
                        ap = inst.ins[0]
                        key = (
                            str(getattr(ap, "memref", None)),
                            getattr(ap, "offset", None),
                            str(getattr(ap, "ap", None)),
                            str(getattr(ap, "dtype", None)),
                            bool(inst.is_transpose),
                            str(inst.perf_mode),
                        )
                        if key == last_key:
                            si = inst.sync_info
                            if si and (si.on_wait or si.on_update):
                                ev = mybir.InstEventSemaphore(
                                    name=f"{inst.name}-ldwe",
                                    engine=inst.engine,
                                    sync_info=si,
                                )
                                nc.register_instruction(ev, overwrite=True)
                                insts[i] = ev
                                i += 1
                            else:
                                del insts[i]
                            n += 1
                            continue
                        last_key = key
                    elif tn in ("InstMatmult", "InstEventSemaphore"):
                        pass
                    else:
                        last_key = None
                i += 1
    return n


def _split_matmul_waits(nc):
    """Walrus encodes fp32/fp32r/transpose matmuls as self-loading LW-struct
    instructions, which accept at most ONE sync-wait command. Tile's wait
    assignment can attach several. Hoist all but one wait of each matmult onto
    standalone EventSemaphore (sequencer) instructions inserted right before
    it on the same engine — semantically identical, codegen-legal."""
    import concourse.mybir as mybir

    skip = ("InstEventSemaphore",)
    n_split = 0
    for fn in nc.m.functions:
        for bb in fn.blocks:
            insts = bb.instructions
            i = 0
            while i < len(insts):
                inst = insts[i]
                if type(inst).__name__ not in skip:
                    si = inst.sync_info
                    waits = list(si.on_wait) if (si and si.on_wait) else []
                    if len(waits) > 1:
                        for j, w in enumerate(waits[:-1]):
                            ev = mybir.InstEventSemaphore(
                                name=f"{inst.name}-hw{j}",
                                engine=inst.engine,
                                sync_info=mybir.SyncInfo(
                                    on_wait=[w], on_update=[]
                                ),
                            )
                            nc.register_instruction(ev, overwrite=True)
                            insts.insert(i, ev)
                            i += 1
                        si.on_wait = [waits[-1]]
                        n_split += 1
                i += 1
    return n_split


_CACHE = {}


def _get_nc():
    if "nc" not in _CACHE:
        import concourse.bass as bass

        nc = bass.Bass("TRN2", target_bir_lowering=False)
        build(nc)
        _CACHE["nc"] = nc
    return _CACHE["nc"]


def _prep_host(x, W, b, ln_w, ln_b):
    """Fold LN affine into weights; pre-transpose W to [L, D_in, D_out];
    cast everything to bf16 and build the replicated bias input."""
    import ml_dtypes

    bf = ml_dtypes.bfloat16
    x = np.asarray(x, dtype=np.float32)
    W = np.asarray(W, dtype=np.float32)
    b = np.asarray(b, dtype=np.float32)
    ln_w = np.asarray(ln_w, dtype=np.float32)
    ln_b = np.asarray(ln_b, dtype=np.float32)

    Wf = W * ln_w[:, None, :]  # scale columns (input dim)
    bfold = b + np.einsum("led,ld->le", W, ln_b)
    WT = np.ascontiguousarray(Wf.transpose(0, 2, 1)).astype(bf)  # [L, Din, E]
    brep = np.ascontiguousarray(
        np.broadcast_to(bfold[:, None, :], (bfold.shape[0], P, D))
    ).astype(bf)
    xb = np.ascontiguousarray(x).astype(bf)
    return xb, WT, brep


def make_in_maps(inputs):
    xb, WT, brep = _prep_host(**inputs)
    return [{"x": xb[i], "wt": WT, "brep": brep} for i in range(B)]


def run(x, W, b, ln_w, ln_b, trace=False):
    from concourse import bass_utils

    in_maps = make_in_maps(dict(x=x, W=W, b=b, ln_w=ln_w, ln_b=ln_b))
    nc = _get_nc()
    res = bass_utils.run_bass_kernel_spmd(
        nc, in_maps, core_ids=list(range(B)), trace=trace
    )
    out = np.stack(
        [np.asarray(res.results[i]["y"], dtype=np.float32) for i in range(B)]
    )
    return out.reshape(B, S, D), res


def kernel(x, W, b, ln_w, ln_b):
    out, _ = run(x, W, b, ln_w, ln_b)
    return out
